# revision 38
# baseline (speedup 1.0000x reference)
"""Trainium2 Bass kernel for a 2-layer LSTM (B=1024, T=512, D=32, H=64) + MLP head.

Strategy (per core, data-parallel over batch: B_local = 128):
  * All tensors "transposed": feature-rows on partitions, batch on free dim.
  * Wavefront over t: iteration k processes layer0 at t=k and layer1 at t=k-1.
    Layer0 state on partitions 0:64, layer1 on 64:128, so every elementwise op
    covers both layers at once.
  * fp16 everywhere except PSUM accumulation (fp32): 1 cycle/row matmuls (4x
    over fp32).  Host emulation: rel err ~1.6e-4 (tolerance 2e-2).
  * One unified h tile HH = [2*h0 ; 2*h1]: a single DVE op updates both
    layers' h, and each gate's recurrent matmul is one K=128, M=128
    instruction (lhsT cols 0:64 layer0, 64:128 layer1).
  * x is preloaded to SBUF in full ([33, (T+1)*BL] fp16, a ones row carrying
    the biases and a trailing zero step so the final wavefront iteration
    still receives layer1's bias) - zero per-step DMA.
  * Two half-batch chains (A: batch 0:64, B: 64:128), each with its own
    4-bank PSUM rotation, pipeline against each other: A's ACT overlaps B's
    matmuls/DVE.  Per-chain banks matter: a reader of a bank waits for that
    bank's whole accumulation group, so sharing one bank would serialize the
    chains.
  * The x-projections run off the critical path (emitted for step k+4); the
    recurrent mmH matmuls accumulate on top (one start, one stop per bank).
  * Gates z = [i|f|g|o] blocks.  One ACT per chain computes s = tanh(0.5*z)
    for all 4 gates; sigma(z) = (tanh(z/2)+1)/2; the g gate's weights/bias
    are pre-doubled so tanh(0.5*2g) = tanh(g).  Keeping every gate on the
    tanh table beats direct sigmoid: one table set, one ACT op per chain.
  * Cell update (state C = 2c, fp16):
      P = (s_f + 1) * C ; Q = (s_i + 1) * s_g ; C' = 0.5*P + Q
      th = tanh(0.5*C') ; HH' = (s_o + 1)*th  (= 2h; h-weights pre-halved)

Measured: 1.213 ms HW exec (3.9x over the 4.72 ms fp32 single-chain
baseline); rel err 1.75e-4.  The cycle (~2.28 us/step) is balanced between
the DVE queue (8 ops), ACT (4 ops, ~72% busy), and the H'->mmH->s tail.
"""

import numpy as np
from contextlib import ExitStack

import concourse.bass as bass
import concourse.bacc as bacc
import concourse.mybir as mybir
import concourse.tile as tile
from concourse.bass_utils import run_bass_kernel_spmd

F32 = mybir.dt.float32
F16 = mybir.dt.float16
AT = mybir.ActivationFunctionType
OP = mybir.AluOpType

B, T, D, H = 1024, 512, 32, 64
N_CORES = 8
BL = B // N_CORES  # 128 batch per core
XROWS = D + 1  # x rows + ones row


def build_nc(t_steps=T, debug=False):
    nc = bacc.Bacc()

    # one extra zero-x step at the end: its projection is exactly the bias,
    # which layer 1 still needs at the final wavefront iteration
    xd = nc.declare_dram_parameter("xh", [XROWS, (t_steps + 1) * BL], F16,
                                   isOutput=False)
    wxd = nc.declare_dram_parameter("wx", [XROWS, 512], F16, isOutput=False)
    whd = nc.declare_dram_parameter("wh", [128, 512], F16, isOutput=False)
    hw1d = nc.declare_dram_parameter("hw1", [65, 32], F16, isOutput=False)
    hw2d = nc.declare_dram_parameter("hw2", [33, 1], F16, isOutput=False)
    yd = nc.declare_dram_parameter("y", [1, BL], F32, isOutput=True)
    if debug:
        dbg_hh = nc.declare_dram_parameter("dbg_hh", [128, BL], F16, isOutput=True)
        dbg_c = nc.declare_dram_parameter("dbg_c", [128, BL], F16, isOutput=True)
        dbg_s = nc.declare_dram_parameter("dbg_s", [128, 512], F16, isOutput=True)

    with tile.TileContext(nc) as tc, ExitStack() as ctx:
        const = ctx.enter_context(tc.tile_pool(name="const", bufs=1))
        st = ctx.enter_context(tc.tile_pool(name="state", bufs=1))
        ps = ctx.enter_context(tc.tile_pool(name="ps", bufs=1, space="PSUM"))

        # ---- resident inputs / weights ----
        # x preload is chunked so the first steps only wait on the first chunk
        X = const.tile([XROWS, (t_steps + 1) * BL], F16)
        nchunk = 16
        csz = ((t_steps + 1 + nchunk - 1) // nchunk) * BL
        for i in range(nchunk):
            lo, hi = i * csz, min((i + 1) * csz, (t_steps + 1) * BL)
            if lo < hi:
                nc.sync.dma_start(X[:, lo:hi], xd[:, lo:hi])
        WX = const.tile([XROWS, 512], F16)
        nc.sync.dma_start(WX[:, :], wxd[:, :])
        WH = const.tile([128, 512], F16)
        nc.sync.dma_start(WH[:, :], whd[:, :])
        hw1 = const.tile([65, 32], F16)
        nc.sync.dma_start(hw1[:, :], hw1d[:, :])
        hw2 = const.tile([33, 1], F16)
        nc.sync.dma_start(hw2[:, :], hw2d[:, :])

        # ---- persistent state ----
        HH = [st.tile([128, BL], F16, name=f"HH_{i}") for i in range(2)]
        S = [st.tile([128, 512], F16, name=f"S_{i}") for i in range(2)]
        C = st.tile([128, BL], F16, name="C")
        P = st.tile([128, BL], F16, name="P")
        Q = st.tile([128, BL], F16, name="Q")
        TH = st.tile([128, BL], F16, name="TH")
        for i in range(2):
            nc.vector.memset(HH[i][:, :], 0.0)
        nc.vector.memset(C[:, :], 0.0)

        # All 8 PSUM banks as one region.  A matmul may not cross a 2KB bank,
        # and each bank gets exactly one start=True (it marks the whole bank
        # pending-zero) and one stop=True per step; groups on different banks
        # may interleave freely.
        Z4 = ps.tile([128, 4096], F32, name="Z4")

        # Two half-batch chains (A: batch 0:64, B: 64:128) pipeline against
        # each other: A's ACT overlaps B's matmuls/DVE.  Each chain owns its
        # own 4-bank rotation (A: banks 0-3, B: banks 4-7) so a reader of one
        # chain's bank never waits on the other chain's accumulation group.
        # Within a bank only cols 0:256 are used: 4 gate blocks of 64.
        HB = BL // 2

        def zcol(c, j, g=0):
            return (c * 4 + j % 4) * 512 + g * HB

        def mm_x(j, stop):
            # x-projection for step j (opens each chain-bank's group)
            for c in range(2):
                rhs = X[:, j * BL + c * HB:j * BL + (c + 1) * HB]
                for g in range(4):
                    o0 = zcol(c, j, g)
                    nc.tensor.matmul(Z4[0:128, o0:o0 + HB],
                                     WX[:, g * 128:(g + 1) * 128], rhs,
                                     start=(g == 0), stop=stop and g == 3)



        # step 0 has no recurrent matmul: its group opens and closes here
        mm_x(0, True)
        for j in range(1, min(4, t_steps + 1)):
            mm_x(j, False)

        for k in range(t_steps + 1):
            l0 = k < t_steps
            l1 = k >= 1
            r0, r1 = (0, 128) if (l0 and l1) else ((0, 64) if l0 else (64, 128))
            s = S[k % 2]

            # recurrent matmuls (accumulate onto the x-projection; every bank
            # incl. the last got an mmX, so never start here)
            if l1:
                for c in range(2):
                    for g in range(4):
                        o0 = zcol(c, k, g)
                        nc.tensor.matmul(
                            Z4[0:128, o0:o0 + HB],
                            WH[:, g * 128:(g + 1) * 128],
                            HH[(k + 1) % 2][0:128, c * HB:(c + 1) * HB],
                            start=False, stop=(g == 3),
                        )

            def act_s(c):
                nc.scalar.activation(
                    s[r0:r1, c * 256:(c + 1) * 256],
                    Z4[r0:r1, zcol(c, k):zcol(c, k) + 256],
                    AT.Tanh, bias=0.0, scale=0.5)

            def pq(c):
                sb = c * 256
                cc = slice(c * HB, (c + 1) * HB)
                # Q = (s_i + 1) * s_g ; P = (s_f + 1) * C_prev
                nc.vector.scalar_tensor_tensor(
                    Q[r0:r1, cc], s[r0:r1, sb:sb + 64], 1.0,
                    s[r0:r1, sb + 128:sb + 192], op0=OP.add, op1=OP.mult)
                nc.vector.scalar_tensor_tensor(
                    P[r0:r1, cc], s[r0:r1, sb + 64:sb + 128], 1.0,
                    C[r0:r1, cc], op0=OP.add, op1=OP.mult)

            def cupd(c):
                cc = slice(c * HB, (c + 1) * HB)
                nc.vector.scalar_tensor_tensor(
                    C[r0:r1, cc], P[r0:r1, cc], 0.5, Q[r0:r1, cc],
                    op0=OP.mult, op1=OP.add)

            def act_th(c):
                cc = slice(c * HB, (c + 1) * HB)
                nc.scalar.activation(TH[r0:r1, cc], C[r0:r1, cc], AT.Tanh,
                                     bias=0.0, scale=0.5)

            def hupd(c):
                cc = slice(c * HB, (c + 1) * HB)
                nc.vector.scalar_tensor_tensor(
                    HH[k % 2][r0:r1, cc], s[r0:r1, c * 256 + 192:c * 256 + 256],
                    1.0, TH[r0:r1, cc], op0=OP.add, op1=OP.mult)

            act_s(0)          # ACT: s_A
            pq(0)             # DVE: P_A, Q_A
            act_s(1)          # ACT: s_B (while DVE works on A)
            cupd(0)           # DVE: C'_A
            act_th(0)         # ACT: th_A
            pq(1)             # DVE: P_B, Q_B (while ACT does th_A)
            cupd(1)           # DVE: C'_B
            act_th(1)         # ACT: th_B
            hupd(0)           # DVE: H'_A
            hupd(1)           # DVE: H'_B

            # x-projection for step k+4, emitted after both s ACTs whose bank
            # it reuses; it executes during this step's DVE/ACT tail
            if k + 4 <= t_steps:
                mm_x(k + 4, False)

        if debug:
            nc.sync.dma_start(dbg_hh[:, :], HH[t_steps % 2][:, :])
            nc.sync.dma_start(dbg_c[:, :], C[:, :])
            nc.sync.dma_start(dbg_s[:, :], S[t_steps % 2][:, :])

        # ---- head: y = W2 @ relu(W1 @ h1 + b1) + b2 ----
        hd = st.tile([65, BL], F16)
        nc.vector.memset(hd[64:65, :], 1.0)
        nc.sync.dma_start(hd[0:64, :], HH[t_steps % 2][64:128, :])
        ph = Z4[0:32, 0:BL]  # bank 0, reused after the loop
        nc.tensor.matmul(ph, hw1[0:65, 0:32], hd[0:65, :],
                         start=True, stop=True)
        hr = st.tile([33, BL], F16)
        nc.vector.memset(hr[32:33, :], 1.0)
        nc.scalar.activation(hr[0:32, :], ph, AT.Relu)
        po = Z4[0:1, 512:512 + BL]  # bank 1
        nc.tensor.matmul(po, hw2[0:33, 0:1], hr[0:33, :],
                         start=True, stop=True)
        ysb = st.tile([1, BL], F32)
        nc.scalar.copy(ysb[0:1, :], po)
        nc.sync.dma_start(yd[:, :], ysb[0:1, :])

    return nc


def prep_weights(Wih0, Whh0, bih0, bhh0, Wih1, Whh1, bih1, bhh1, W1, b1, W2, b2):
    """Host-side weight re-layout, fp16.  Gate order i,f,g,o.

    Scalings (exact powers of two):
      * h-input columns halved (state is stored as 2*h),
      * g gate's whole block (weights + bias) doubled so tanh(0.5*z) yields
        exactly tanh(g).
    """
    f32, f16 = np.float32, np.float16
    bias0 = (bih0 + bhh0).astype(f32)
    bias1 = (bih1 + bhh1).astype(f32)
    wx = np.zeros((XROWS, 512), f32)
    wh = np.zeros((128, 512), f32)
    for g in range(4):
        rs = slice(g * 64, (g + 1) * 64)
        sc = 2.0 if g == 2 else 1.0
        c0 = slice(g * 128, g * 128 + 64)        # layer0 cols
        c1 = slice(g * 128 + 64, g * 128 + 128)  # layer1 cols
        wx[0:32, c0] = Wih0[rs, :].T * sc
        wx[32, c0] = bias0[rs] * sc
        wx[32, c1] = bias1[rs] * sc
        wh[0:64, c0] = Whh0[rs, :].T * (0.5 * sc)
        wh[0:64, c1] = Wih1[rs, :].T * (0.5 * sc)
        wh[64:128, c1] = Whh1[rs, :].T * (0.5 * sc)
    hw1 = np.zeros((65, 32), f32)
    hw1[0:64, :] = W1.T * 0.5
    hw1[64, :] = b1
    hw2 = np.zeros((33, 1), f32)
    hw2[0:32, :] = W2.T
    hw2[32, :] = b2
    return wx.astype(f16), wh.astype(f16), hw1.astype(f16), hw2.astype(f16)


_NC_CACHE = {}


def _get_nc(t_steps, debug=False):
    key = (t_steps, debug)
    if key not in _NC_CACHE:
        nc = build_nc(t_steps, debug)
        if not nc.is_finalized():
            nc.finalize()
        _NC_CACHE[key] = nc
    return _NC_CACHE[key]


def run(x, weights, t_steps=T, trace=False, debug=False):
    """x: [B, t_steps, D] float32; weights: tuple from prep_weights."""
    wx, wh, hw1, hw2 = weights
    nc = _get_nc(t_steps, debug)
    # [B, T, D] -> [D, T, B] -> per-core [XROWS, T*BL] with ones row
    xs = np.ascontiguousarray(x.transpose(2, 1, 0)).astype(np.float16)
    in_maps = []
    for c in range(N_CORES):
        xh = np.zeros((XROWS, (t_steps + 1) * BL), np.float16)
        xh[0:D, 0:t_steps * BL] = xs[:, :, c * BL:(c + 1) * BL].reshape(
            D, t_steps * BL)
        xh[D, :] = np.float16(1.0)
        in_maps.append({
            "xh": xh, "wx": wx, "wh": wh, "hw1": hw1, "hw2": hw2,
        })
    res = run_bass_kernel_spmd(nc, in_maps, core_ids=list(range(N_CORES)),
                               trace=trace)
    y = np.concatenate([res.results[c]["y"][0] for c in range(N_CORES)])
    return y, res


def kernel(x, Wih0, Whh0, bih0, bhh0, Wih1, Whh1, bih1, bhh1, W1, b1, W2, b2):
    weights = prep_weights(
        np.asarray(Wih0, np.float32), np.asarray(Whh0, np.float32),
        np.asarray(bih0, np.float32), np.asarray(bhh0, np.float32),
        np.asarray(Wih1, np.float32), np.asarray(Whh1, np.float32),
        np.asarray(bih1, np.float32), np.asarray(bhh1, np.float32),
        np.asarray(W1, np.float32), np.asarray(b1, np.float32),
        np.asarray(W2, np.float32), np.asarray(b2, np.float32),
    )
    y, _ = run(np.asarray(x, np.float32), weights)
    return y


# revision 39
# speedup vs baseline: 1.1970x; 1.1970x over previous
"""Trainium2 Bass kernel for a 2-layer LSTM (B=1024, T=512, D=32, H=64) + MLP head.

Strategy (per core, data-parallel over batch: B_local = 128):
  * All tensors "transposed": feature-rows on partitions, batch on free dim.
  * Wavefront over t: iteration k processes layer0 at t=k and layer1 at t=k-1.
    Layer0 state on partitions 0:64, layer1 on 64:128, so every elementwise op
    covers both layers at once.
  * fp16 everywhere except PSUM accumulation (fp32): 1 cycle/row matmuls (4x
    over fp32).  Host emulation: rel err ~1.6e-4 (tolerance 2e-2).
  * One unified h tile HH = [2*h0 ; 2*h1]: a single DVE op updates both
    layers' h, and each gate's recurrent matmul is one K=128, M=128
    instruction (lhsT cols 0:64 layer0, 64:128 layer1).
  * x is preloaded to SBUF in full ([33, (T+1)*BL] fp16, a ones row carrying
    the biases and a trailing zero step so the final wavefront iteration
    still receives layer1's bias) - zero per-step DMA.
  * Two half-batch chains (A: batch 0:64, B: 64:128), each with its own
    4-bank PSUM rotation, pipeline against each other: A's ACT overlaps B's
    matmuls/DVE.  Per-chain banks matter: a reader of a bank waits for that
    bank's whole accumulation group, so sharing one bank would serialize the
    chains.
  * The x-projections run off the critical path (emitted for step k+4); the
    recurrent mmH matmuls accumulate on top (one start, one stop per bank).
  * Gates z = [i|f|g|o] blocks.  One ACT per chain computes s = tanh(0.5*z)
    for all 4 gates; sigma(z) = (tanh(z/2)+1)/2; the g gate's weights/bias
    are pre-doubled so tanh(0.5*2g) = tanh(g).  Keeping every gate on the
    tanh table beats direct sigmoid: one table set, one ACT op per chain.
  * Cell update (state C = 2c, fp16):
      P = (s_f + 1) * C ; Q = (s_i + 1) * s_g ; C' = 0.5*P + Q
      th = tanh(0.5*C') ; HH' = (s_o + 1)*th  (= 2h; h-weights pre-halved)

Measured: 1.213 ms HW exec (3.9x over the 4.72 ms fp32 single-chain
baseline); rel err 1.75e-4.  The cycle (~2.28 us/step) is balanced between
the DVE queue (8 ops), ACT (4 ops, ~72% busy), and the H'->mmH->s tail.
"""

import numpy as np
from contextlib import ExitStack

import concourse.bass as bass
import concourse.bacc as bacc
import concourse.mybir as mybir
import concourse.tile as tile
from concourse.bass_utils import run_bass_kernel_spmd

F32 = mybir.dt.float32
F16 = mybir.dt.float16
AT = mybir.ActivationFunctionType
OP = mybir.AluOpType

B, T, D, H = 1024, 512, 32, 64
N_CORES = 8
BL = B // N_CORES  # 128 batch per core
XROWS = D + 1  # x rows + ones row


def build_nc(t_steps=T, debug=False):
    nc = bacc.Bacc()

    # one extra zero-x step at the end: its projection is exactly the bias,
    # which layer 1 still needs at the final wavefront iteration
    xd = nc.declare_dram_parameter("xh", [XROWS, (t_steps + 1) * BL], F16,
                                   isOutput=False)
    wxd = nc.declare_dram_parameter("wx", [XROWS, 512], F16, isOutput=False)
    whd = nc.declare_dram_parameter("wh", [128, 512], F16, isOutput=False)
    hw1d = nc.declare_dram_parameter("hw1", [65, 32], F16, isOutput=False)
    hw2d = nc.declare_dram_parameter("hw2", [33, 1], F16, isOutput=False)
    yd = nc.declare_dram_parameter("y", [1, BL], F32, isOutput=True)
    if debug:
        dbg_hh = nc.declare_dram_parameter("dbg_hh", [128, BL], F16, isOutput=True)
        dbg_c = nc.declare_dram_parameter("dbg_c", [128, BL], F16, isOutput=True)
        dbg_s = nc.declare_dram_parameter("dbg_s", [128, 512], F16, isOutput=True)

    with tile.TileContext(nc) as tc, ExitStack() as ctx:
        const = ctx.enter_context(tc.tile_pool(name="const", bufs=1))
        st = ctx.enter_context(tc.tile_pool(name="state", bufs=1))
        ps = ctx.enter_context(tc.tile_pool(name="ps", bufs=1, space="PSUM"))

        # ---- resident inputs / weights ----
        # x preload is chunked so the first steps only wait on the first chunk
        X = const.tile([XROWS, (t_steps + 1) * BL], F16)
        nchunk = 16
        csz = ((t_steps + 1 + nchunk - 1) // nchunk) * BL
        for i in range(nchunk):
            lo, hi = i * csz, min((i + 1) * csz, (t_steps + 1) * BL)
            if lo < hi:
                nc.sync.dma_start(X[:, lo:hi], xd[:, lo:hi])
        WX = const.tile([XROWS, 512], F16)
        nc.sync.dma_start(WX[:, :], wxd[:, :])
        WH = const.tile([128, 512], F16)
        nc.sync.dma_start(WH[:, :], whd[:, :])
        hw1 = const.tile([65, 32], F16)
        nc.sync.dma_start(hw1[:, :], hw1d[:, :])
        hw2 = const.tile([33, 1], F16)
        nc.sync.dma_start(hw2[:, :], hw2d[:, :])

        # ---- persistent state ----
        HH = [st.tile([128, BL], F16, name=f"HH_{i}") for i in range(2)]
        S = [st.tile([128, 512], F16, name=f"S_{i}") for i in range(2)]
        C = st.tile([128, BL], F16, name="C")
        P = st.tile([128, BL], F16, name="P")
        Q = st.tile([128, BL], F16, name="Q")
        TH = st.tile([128, BL], F16, name="TH")
        for i in range(2):
            nc.vector.memset(HH[i][:, :], 0.0)
        nc.vector.memset(C[:, :], 0.0)

        # All 8 PSUM banks as one region.  A matmul may not cross a 2KB bank,
        # and each bank gets exactly one start=True (it marks the whole bank
        # pending-zero) and one stop=True per step; groups on different banks
        # may interleave freely.
        Z4 = ps.tile([128, 4096], F32, name="Z4")

        # Two half-batch chains (A: batch 0:64, B: 64:128) pipeline against
        # each other: A's ACT overlaps B's matmuls/DVE.  Each chain owns its
        # own 4-bank rotation (A: banks 0-3, B: banks 4-7) so a reader of one
        # chain's bank never waits on the other chain's accumulation group.
        # Within a bank only cols 0:256 are used: 4 gate blocks of 64.
        HB = BL // 2

        def zcol(c, j, g=0):
            return (c * 4 + j % 4) * 512 + g * HB

        def mm_x(j, stop):
            # x-projection for step j (opens each chain-bank's group)
            for c in range(2):
                rhs = X[:, j * BL + c * HB:j * BL + (c + 1) * HB]
                for g in range(4):
                    o0 = zcol(c, j, g)
                    nc.tensor.matmul(Z4[0:128, o0:o0 + HB],
                                     WX[:, g * 128:(g + 1) * 128], rhs,
                                     start=(g == 0), stop=stop and g == 3)



        # step 0 has no recurrent matmul: its group opens and closes here
        mm_x(0, True)
        for j in range(1, min(4, t_steps + 1)):
            mm_x(j, False)

        for k in range(t_steps + 1):
            l0 = k < t_steps
            l1 = k >= 1
            r0, r1 = (0, 128) if (l0 and l1) else ((0, 64) if l0 else (64, 128))
            s = S[k % 2]

            # recurrent matmuls (accumulate onto the x-projection; every bank
            # incl. the last got an mmX, so never start here)
            if l1:
                for c in range(2):
                    for g in range(4):
                        o0 = zcol(c, k, g)
                        nc.tensor.matmul(
                            Z4[0:128, o0:o0 + HB],
                            WH[:, g * 128:(g + 1) * 128],
                            HH[(k + 1) % 2][0:128, c * HB:(c + 1) * HB],
                            start=False, stop=(g == 3),
                        )

            def act_s(c):
                nc.scalar.activation(
                    s[r0:r1, c * 256:(c + 1) * 256],
                    Z4[r0:r1, zcol(c, k):zcol(c, k) + 256],
                    AT.Tanh, bias=0.0, scale=0.5)

            def pq(c):
                sb = c * 256
                cc = slice(c * HB, (c + 1) * HB)
                # P = (s_f + 1) * C_prev ; Q = (s_i + 1) * s_g
                nc.vector.scalar_tensor_tensor(
                    P[r0:r1, cc], s[r0:r1, sb + 64:sb + 128], 1.0,
                    C[r0:r1, cc], op0=OP.add, op1=OP.mult)
                nc.vector.scalar_tensor_tensor(
                    Q[r0:r1, cc], s[r0:r1, sb:sb + 64], 1.0,
                    s[r0:r1, sb + 128:sb + 192], op0=OP.add, op1=OP.mult)

            def cupd(c):
                cc = slice(c * HB, (c + 1) * HB)
                nc.vector.scalar_tensor_tensor(
                    C[r0:r1, cc], P[r0:r1, cc], 0.5, Q[r0:r1, cc],
                    op0=OP.mult, op1=OP.add)

            def act_th(c):
                cc = slice(c * HB, (c + 1) * HB)
                nc.scalar.activation(TH[r0:r1, cc], C[r0:r1, cc], AT.Tanh,
                                     bias=0.0, scale=0.5)

            def hupd(c):
                cc = slice(c * HB, (c + 1) * HB)
                nc.vector.scalar_tensor_tensor(
                    HH[k % 2][r0:r1, cc], s[r0:r1, c * 256 + 192:c * 256 + 256],
                    1.0, TH[r0:r1, cc], op0=OP.add, op1=OP.mult)

            act_s(0)          # ACT: s_A
            pq(0)             # DVE: P_A, Q_A
            act_s(1)          # ACT: s_B (while DVE works on A)
            cupd(0)           # DVE: C'_A
            act_th(0)         # ACT: th_A
            pq(1)             # DVE: P_B, Q_B (while ACT does th_A)
            cupd(1)           # DVE: C'_B
            act_th(1)         # ACT: th_B
            hupd(0)           # DVE: H'_A
            hupd(1)           # DVE: H'_B

            # x-projection for step k+4, emitted after both s ACTs whose bank
            # it reuses; it executes during this step's DVE/ACT tail
            if k + 4 <= t_steps:
                mm_x(k + 4, False)

        if debug:
            nc.sync.dma_start(dbg_hh[:, :], HH[t_steps % 2][:, :])
            nc.sync.dma_start(dbg_c[:, :], C[:, :])
            nc.sync.dma_start(dbg_s[:, :], S[t_steps % 2][:, :])

        # ---- head: y = W2 @ relu(W1 @ h1 + b1) + b2 ----
        hd = st.tile([65, BL], F16)
        nc.vector.memset(hd[64:65, :], 1.0)
        nc.sync.dma_start(hd[0:64, :], HH[t_steps % 2][64:128, :])
        ph = Z4[0:32, 0:BL]  # bank 0, reused after the loop
        nc.tensor.matmul(ph, hw1[0:65, 0:32], hd[0:65, :],
                         start=True, stop=True)
        hr = st.tile([33, BL], F16)
        nc.vector.memset(hr[32:33, :], 1.0)
        nc.scalar.activation(hr[0:32, :], ph, AT.Relu)
        po = Z4[0:1, 512:512 + BL]  # bank 1
        nc.tensor.matmul(po, hw2[0:33, 0:1], hr[0:33, :],
                         start=True, stop=True)
        ysb = st.tile([1, BL], F32)
        nc.scalar.copy(ysb[0:1, :], po)
        nc.sync.dma_start(yd[:, :], ysb[0:1, :])

    return nc


def prep_weights(Wih0, Whh0, bih0, bhh0, Wih1, Whh1, bih1, bhh1, W1, b1, W2, b2):
    """Host-side weight re-layout, fp16.  Gate order i,f,g,o.

    Scalings (exact powers of two):
      * h-input columns halved (state is stored as 2*h),
      * g gate's whole block (weights + bias) doubled so tanh(0.5*z) yields
        exactly tanh(g).
    """
    f32, f16 = np.float32, np.float16
    bias0 = (bih0 + bhh0).astype(f32)
    bias1 = (bih1 + bhh1).astype(f32)
    wx = np.zeros((XROWS, 512), f32)
    wh = np.zeros((128, 512), f32)
    for g in range(4):
        rs = slice(g * 64, (g + 1) * 64)
        sc = 2.0 if g == 2 else 1.0
        c0 = slice(g * 128, g * 128 + 64)        # layer0 cols
        c1 = slice(g * 128 + 64, g * 128 + 128)  # layer1 cols
        wx[0:32, c0] = Wih0[rs, :].T * sc
        wx[32, c0] = bias0[rs] * sc
        wx[32, c1] = bias1[rs] * sc
        wh[0:64, c0] = Whh0[rs, :].T * (0.5 * sc)
        wh[0:64, c1] = Wih1[rs, :].T * (0.5 * sc)
        wh[64:128, c1] = Whh1[rs, :].T * (0.5 * sc)
    hw1 = np.zeros((65, 32), f32)
    hw1[0:64, :] = W1.T * 0.5
    hw1[64, :] = b1
    hw2 = np.zeros((33, 1), f32)
    hw2[0:32, :] = W2.T
    hw2[32, :] = b2
    return wx.astype(f16), wh.astype(f16), hw1.astype(f16), hw2.astype(f16)


_NC_CACHE = {}


def _get_nc(t_steps, debug=False):
    key = (t_steps, debug)
    if key not in _NC_CACHE:
        nc = build_nc(t_steps, debug)
        if not nc.is_finalized():
            nc.finalize()
        _NC_CACHE[key] = nc
    return _NC_CACHE[key]


def run(x, weights, t_steps=T, trace=False, debug=False):
    """x: [B, t_steps, D] float32; weights: tuple from prep_weights."""
    wx, wh, hw1, hw2 = weights
    nc = _get_nc(t_steps, debug)
    # [B, T, D] -> [D, T, B] -> per-core [XROWS, T*BL] with ones row
    xs = np.ascontiguousarray(x.transpose(2, 1, 0)).astype(np.float16)
    in_maps = []
    for c in range(N_CORES):
        xh = np.zeros((XROWS, (t_steps + 1) * BL), np.float16)
        xh[0:D, 0:t_steps * BL] = xs[:, :, c * BL:(c + 1) * BL].reshape(
            D, t_steps * BL)
        xh[D, :] = np.float16(1.0)
        in_maps.append({
            "xh": xh, "wx": wx, "wh": wh, "hw1": hw1, "hw2": hw2,
        })
    res = run_bass_kernel_spmd(nc, in_maps, core_ids=list(range(N_CORES)),
                               trace=trace)
    y = np.concatenate([res.results[c]["y"][0] for c in range(N_CORES)])
    return y, res


def kernel(x, Wih0, Whh0, bih0, bhh0, Wih1, Whh1, bih1, bhh1, W1, b1, W2, b2):
    weights = prep_weights(
        np.asarray(Wih0, np.float32), np.asarray(Whh0, np.float32),
        np.asarray(bih0, np.float32), np.asarray(bhh0, np.float32),
        np.asarray(Wih1, np.float32), np.asarray(Whh1, np.float32),
        np.asarray(bih1, np.float32), np.asarray(bhh1, np.float32),
        np.asarray(W1, np.float32), np.asarray(b1, np.float32),
        np.asarray(W2, np.float32), np.asarray(b2, np.float32),
    )
    y, _ = run(np.asarray(x, np.float32), weights)
    return y
